# revision 4
# baseline (speedup 1.0000x reference)
"""AgentAttention Trainium2 kernel (B=64, N=1024, C=512, M=16 agents) on 8 NeuronCores.

Data-parallel over batch: each core processes 8 batch elements. No collectives.

Math (per batch element, reference semantics):
    Q = x@Wq.T+bq ; K = x@Wk.T+bk ; V = x@Wv.T+bv
    A = group-mean of Q over 64-token groups          -> [16, C]
    S1 = softmax(Q A^T / sqrt(C), axis=agents)        -> [N, 16]
    S2 = softmax(A K^T / sqrt(C), axis=tokens)        -> [16, N]
    out = (S1 @ (S2 @ V)) @ Wo.T + bo

Folds used on device:
    - bv never materialized: S1/S2 rows sum to 1 => out += (Wo@bv + bo) == b'
    - A uses group-SUM; the 1/64 is folded into the logit scale s = 1/(64*sqrt(C))
    - stage-1 softmax normalizer r1 applied as a per-row scale on the final
      output tile; stage-2 normalizer r2 applied as per-row scale when evicting
      the S2@V accumulation from PSUM.
All matmuls in bf16 with fp32 PSUM accumulation.
"""

import sys
import os

if "/opt/trn_rl_repo" not in sys.path:
    sys.path.insert(0, "/opt/trn_rl_repo")

import numpy as np
import ml_dtypes

import concourse.bass as bass
import concourse.mybir as mybir
import concourse.tile as tile
from concourse import bacc
from concourse.bass import ts, ds
from concourse.bass_utils import run_bass_kernel_spmd
from concourse.masks import make_identity

BF16 = mybir.dt.bfloat16
F32 = mybir.dt.float32

N_CORES = 8
B = 64
B_LOC = B // N_CORES  # 8 batches per core
N = 1024              # tokens
C = 512               # channels
M = 16                # agents
G = N // M            # 64-token pooling groups
P = 128
ND = C // P           # 4 channel chunks
NN = N // P           # 8 token chunks of 128
NI = N // 512         # 2 token chunks of 512
SCALE = 1.0 / (G * np.sqrt(C))  # logit scale (1/64 pooling fold included)

# test harness may override (e.g. {"trace": True, "tmpdir": ...})
_RUN_KWARGS = {}
_LAST_RESULTS = None


def _build_program():
    nc = bacc.Bacc("TRN2", target_bir_lowering=False, debug=False,
                   num_devices=N_CORES)

    xT_d = nc.dram_tensor("xT", [B_LOC, C, N], BF16, kind="ExternalInput")
    wq_d = nc.dram_tensor("wqT", [C, C], BF16, kind="ExternalInput")
    wk_d = nc.dram_tensor("wkT", [C, C], BF16, kind="ExternalInput")
    wv_d = nc.dram_tensor("wvT", [C, C], BF16, kind="ExternalInput")
    wo_d = nc.dram_tensor("woT", [C, C], BF16, kind="ExternalInput")
    bq_d = nc.dram_tensor("bq", [C], F32, kind="ExternalInput")
    bk_d = nc.dram_tensor("bk", [C], F32, kind="ExternalInput")
    bp_d = nc.dram_tensor("bp", [P, C], F32, kind="ExternalInput")
    out_d = nc.dram_tensor("out", [B_LOC, N, C], F32, kind="ExternalOutput")

    with tile.TileContext(nc) as tc:
        with (
            tc.tile_pool(name="const", bufs=1) as const,
            tc.tile_pool(name="px", bufs=2) as px,
            tc.tile_pool(name="pq", bufs=2) as pq,
            tc.tile_pool(name="pk", bufs=2) as pk,
            tc.tile_pool(name="pv", bufs=2) as pv,
            tc.tile_pool(name="py", bufs=2) as py,
            tc.tile_pool(name="psmall", bufs=2) as psmall,
            tc.tile_pool(name="pout", bufs=3) as pout,
            tc.tile_pool(name="ps_mm", bufs=2, space="PSUM") as ps_mm,
            tc.tile_pool(name="ps_log", bufs=3, space="PSUM") as ps_log,
            tc.tile_pool(name="ps_tiny", bufs=1, space="PSUM") as ps_tiny,
            tc.tile_pool(name="ps_tr", bufs=2, space="PSUM") as ps_tr,
        ):
            # ---- constants / weights (loaded once) ----
            wq_s = const.tile([P, ND, C], BF16)
            wk_s = const.tile([P, ND, C], BF16)
            wv_s = const.tile([P, ND, C], BF16)
            wo_s = const.tile([P, ND, C], BF16)
            nc.sync.dma_start(wq_s[:], wq_d.ap().rearrange("(o p) d -> p o d", p=P))
            nc.sync.dma_start(wk_s[:], wk_d.ap().rearrange("(o p) d -> p o d", p=P))
            nc.sync.dma_start(wv_s[:], wv_d.ap().rearrange("(o p) d -> p o d", p=P))
            nc.sync.dma_start(wo_s[:], wo_d.ap().rearrange("(o p) d -> p o d", p=P))
            bq_s = const.tile([P, ND], F32)
            bk_s = const.tile([P, ND], F32)
            nc.sync.dma_start(bq_s[:], bq_d.ap().rearrange("(o p) -> p o", p=P))
            nc.sync.dma_start(bk_s[:], bk_d.ap().rearrange("(o p) -> p o", p=P))
            bp_s = const.tile([P, C], F32)
            nc.sync.dma_start(bp_s[:], bp_d.ap())
            ident = const.tile([P, P], BF16)
            make_identity(nc, ident)
            ones16 = const.tile([M, 1], BF16)
            nc.vector.memset(ones16[:], 1.0)
            zb16 = const.tile([M, 1], F32)
            nc.vector.memset(zb16[:], 0.0)

            for b in range(B_LOC):
                # ---- load x^T for this batch: [128, 4(c-chunk), 1024] bf16
                xt = px.tile([P, ND, N], BF16, tag="xt")
                nc.sync.dma_start(
                    xt[:], xT_d.ap()[b].rearrange("(o p) n -> p o n", p=P))

                # ---- Q^T = Wq^T-chunks.T @ x^T  (+bq), K^T likewise
                qt = pq.tile([P, ND, N], BF16, tag="qt")
                kt = pk.tile([P, ND, N], BF16, tag="kt")
                asum_f = psmall.tile([P, ND, M], F32, tag="asum_f")
                for (w_s, b_s, dst) in ((wq_s, bq_s, qt), (wk_s, bk_s, kt)):
                    for d in range(ND):
                        for ni in range(NI):
                            ps = ps_mm.tile([P, 512], F32, tag="mm")
                            for c in range(ND):
                                nc.tensor.matmul(
                                    ps[:], w_s[:, c, ds(d * P, P)],
                                    xt[:, c, ts(ni, 512)],
                                    start=(c == 0), stop=(c == ND - 1))
                            # evict with per-partition bias add, cast to bf16
                            nc.scalar.activation(
                                dst[:, d, ts(ni, 512)], ps[:],
                                mybir.ActivationFunctionType.Identity,
                                bias=b_s[:, d:d + 1])

                # ---- agent pooling: group-sum of Q^T along tokens -> [128,4,16]
                for d in range(ND):
                    nc.vector.reduce_sum(
                        asum_f[:, d, :],
                        qt[:, d, :].rearrange("p (g w) -> p g w", w=G),
                        axis=mybir.AxisListType.X)
                asum_b = psmall.tile([P, ND, M], BF16, tag="asum_b")
                nc.vector.tensor_copy(asum_b[:], asum_f[:])

                # ---- V = x^T-chunks.T @ Wv^T : [n-chunk, 512] (no bias; folded)
                vt = pv.tile([P, NN, C], BF16, tag="vt")
                for n in range(NN):
                    ps = ps_mm.tile([P, 512], F32, tag="mm")
                    for c in range(ND):
                        nc.tensor.matmul(
                            ps[:], xt[:, c, ts(n, P)], wv_s[:, c, :],
                            start=(c == 0), stop=(c == ND - 1))
                    nc.vector.tensor_copy(vt[:, n, :], ps[:])

                # ---- stage-1 logits L1^T = Asum^T.T @ Q^T -> E1^T=[16,1024] bf16
                e1t = psmall.tile([M, N], BF16, tag="e1t")
                for ni in range(NI):
                    psl = ps_log.tile([M, 512], F32, tag="log")
                    for d in range(ND):
                        nc.tensor.matmul(
                            psl[:], asum_b[:, d, :], qt[:, d, ts(ni, 512)],
                            start=(d == 0), stop=(d == ND - 1))
                    nc.scalar.activation(
                        e1t[:, ts(ni, 512)], psl[:],
                        mybir.ActivationFunctionType.Exp,
                        bias=zb16[:], scale=float(SCALE))

                # ---- stage-1 denominators: r1[n] = sum_m E1^T[m,n] -> [128,8]
                r_ps = ps_tiny.tile([P, NN], F32, tag="tiny")
                for n in range(NN):
                    nc.tensor.matmul(
                        r_ps[:, n:n + 1], e1t[:, ts(n, P)], ones16[:],
                        start=True, stop=True)
                r_inv = psmall.tile([P, NN], F32, tag="r_inv")
                nc.vector.reciprocal(r_inv[:], r_ps[:])

                # ---- stage-2 logits L2 = Asum^T.T @ K^T -> E2=[16,1024] bf16
                e2 = psmall.tile([M, N], BF16, tag="e2")
                d2 = psmall.tile([M, NI], F32, tag="d2")
                for ni in range(NI):
                    psl = ps_log.tile([M, 512], F32, tag="log")
                    for d in range(ND):
                        nc.tensor.matmul(
                            psl[:], asum_b[:, d, :], kt[:, d, ts(ni, 512)],
                            start=(d == 0), stop=(d == ND - 1))
                    nc.scalar.activation(
                        e2[:, ts(ni, 512)], psl[:],
                        mybir.ActivationFunctionType.Exp,
                        bias=zb16[:], scale=float(SCALE),
                        accum_out=d2[:, ni:ni + 1])
                d2s = psmall.tile([M, 1], F32, tag="d2s")
                nc.vector.tensor_add(d2s[:], d2[:, 0:1], d2[:, 1:2])
                r2 = psmall.tile([M, 1], F32, tag="r2")
                nc.vector.reciprocal(r2[:], d2s[:])

                # ---- transpose E2 -> E2^T chunks [128, 16]
                e2t = psmall.tile([P, NN, M], BF16, tag="e2t")
                for n in range(NN):
                    pst = ps_tr.tile([P, M], BF16, tag="tiny_t")
                    nc.tensor.transpose(pst[:], e2[:, ts(n, P)], ident[:M, :M])
                    nc.vector.tensor_copy(e2t[:, n, :], pst[:])

                # ---- AF = (E2 @ V) * r2 (stage-2 softmax normalization fused)
                psa = ps_log.tile([M, 512], F32, tag="log")
                for n in range(NN):
                    nc.tensor.matmul(
                        psa[:], e2t[:, n, :], vt[:, n, :],
                        start=(n == 0), stop=(n == NN - 1))
                af = psmall.tile([M, C], BF16, tag="af")
                nc.scalar.activation(
                    af[:], psa[:], mybir.ActivationFunctionType.Copy,
                    scale=r2[:])

                # ---- Y0^T = AF-chunks.T @ E1^T : [c-chunk(128), 1024] bf16
                yt = py.tile([P, ND, N], BF16, tag="yt")
                for d in range(ND):
                    for ni in range(NI):
                        ps = ps_mm.tile([P, 512], F32, tag="mm")
                        nc.tensor.matmul(
                            ps[:], af[:, ts(d, P)], e1t[:, ts(ni, 512)],
                            start=True, stop=True)
                        nc.vector.tensor_copy(yt[:, d, ts(ni, 512)], ps[:])

                # ---- out = (Y0^T-chunks.T @ Wo^T) * r1 + b'
                for n in range(NN):
                    ps = ps_mm.tile([P, 512], F32, tag="mm")
                    for d in range(ND):
                        nc.tensor.matmul(
                            ps[:], yt[:, d, ts(n, P)], wo_s[:, d, :],
                            start=(d == 0), stop=(d == ND - 1))
                    o_s = pout.tile([P, C], F32, tag="o")
                    nc.scalar.activation(
                        o_s[:], ps[:], mybir.ActivationFunctionType.Copy,
                        scale=r_inv[:, n:n + 1])
                    nc.vector.tensor_add(o_s[:], o_s[:], bp_s[:])
                    nc.sync.dma_start(out_d.ap()[b][ts(n, P), :], o_s[:])

    nc.compile()
    return nc


def _prep_inputs(x, Wq, bq, Wk, bk, Wv, bv, Wo, bo):
    bf = ml_dtypes.bfloat16
    xT = np.ascontiguousarray(np.asarray(x, np.float32).transpose(0, 2, 1)).astype(bf)
    shared = {
        "wqT": np.ascontiguousarray(np.asarray(Wq, np.float32).T).astype(bf),
        "wkT": np.ascontiguousarray(np.asarray(Wk, np.float32).T).astype(bf),
        "wvT": np.ascontiguousarray(np.asarray(Wv, np.float32).T).astype(bf),
        "woT": np.ascontiguousarray(np.asarray(Wo, np.float32).T).astype(bf),
        "bq": np.asarray(bq, np.float32),
        "bk": np.asarray(bk, np.float32),
    }
    bprime = (np.asarray(bo, np.float64)
              + np.asarray(Wo, np.float64) @ np.asarray(bv, np.float64))
    shared["bp"] = np.tile(bprime.astype(np.float32), (P, 1))
    in_maps = []
    for c in range(N_CORES):
        m = dict(shared)
        m["xT"] = np.ascontiguousarray(xT[c * B_LOC:(c + 1) * B_LOC])
        in_maps.append(m)
    return in_maps


def kernel(x, Wq, bq, Wk, bk, Wv, bv, Wo, bo):
    global _LAST_RESULTS
    nc = _build_program()
    in_maps = _prep_inputs(x, Wq, bq, Wk, bk, Wv, bv, Wo, bo)
    res = run_bass_kernel_spmd(nc, in_maps, list(range(N_CORES)), **_RUN_KWARGS)
    _LAST_RESULTS = res
    out = np.concatenate([res.results[i]["out"] for i in range(N_CORES)], axis=0)
    return out.astype(np.float32)
